# revision 5
# baseline (speedup 1.0000x reference)
"""Trainium2 Bass kernel for nn_MemoryModel (scatter_memory, 8 cores).

Math (per stage): the 8-point Gauss-Legendre quadrature over matrix
polynomials collapses algebraically:

  LHS_k = I - REG*t_k*D + REG^2*(t_k*D@L + t_k^2/2*D@D)      (D=delta_L, L=L_agg)
  integral = sum_k (LHS_k @ (w_k*V)) * exp(dA*t_k)
           = V*S0 - REG*U*S1 + REG^2*P*S1 + REG^2/2*Q*S2
  with V = X - REG*(L@X),  U = D@V, W1 = L@V, P = D@W1, Q = D@U
  and moments S_j = sum_k w_k t_k^j exp(dA t_k)   (elementwise [n,H])
  As_bar @ M = M - REG*(D@M) + REG^2*(D@(L@M)) + REG^2/2*(D@(D@M))

Sharding: H=128 column-sharded 8 ways (16 cols/core); [1024,1024] operators
replicated (bf16, q-chunk packed); per-node small pipeline in transposed
land; heavy chains per-core in node-packed layout [128p, 8q, 16h].
m1|m2 combined into one [100000,32] bf16 table, gathered with 8 indirect
DMAs. gelu(c1) applied per-shard pre-AllGather; the AllGather carries u2
(bf16) and is warmed by a dummy collective at kernel start. DMA instruction
count minimized (consts grouped into 2 tensors; operators 1 DMA each across
the SP and Activation HWDGE queues) - each DMA_DIRECT2D costs ~0.7us issue.
Stage tails (combine+gelu+transpose) run per-q inside the D2 callbacks to
pipeline with the matmul stream.
"""
import os
import sys

import numpy as np

for _p in ("/opt/trn_rl_repo", "/root/.axon_site/_ro/trn_rl_repo"):
    if os.path.isdir(_p) and _p not in sys.path:
        sys.path.insert(0, _p)

import ml_dtypes  # noqa: E402
import concourse.bass as bass  # noqa: E402
import concourse.bacc as bacc  # noqa: E402
import concourse.mybir as mybir  # noqa: E402
import concourse.tile as tile  # noqa: E402
from concourse.bass_utils import run_bass_kernel_spmd  # noqa: E402

F32 = mybir.dt.float32
BF16 = mybir.dt.bfloat16
I32 = mybir.dt.int32
AF = mybir.ActivationFunctionType
OP = mybir.AluOpType
BF = ml_dtypes.bfloat16

NA, H, DIN, E, NN, ED = 1024, 128, 172, 256, 100000, 1
KD = DIN + 2 * ED  # 174
KDP = 256  # padded contraction for the tune matmul
REG = 0.1
REG2 = REG * REG
NCORES = 8
HS = 16  # H columns per core
NQ = 8  # node tiles (1024/128)

_gl_nodes = [-0.1834346424956498, -0.525532409916329, -0.7966664774136267,
             -0.9602898564975363, 0.1834346424956498, 0.525532409916329,
             0.7966664774136267, 0.9602898564975363]
_gl_w = [0.362683783378362, 0.3137066458778873, 0.2223810344533745,
         0.1012285362903763] * 2
T_NODES = [0.5 * (x + 1.0) for x in _gl_nodes]
T_W = [0.5 * w for w in _gl_w]

# f32 const-group free offsets
F_BT, F_R1, F_R2, F_BB1, F_BB2, F_ACT, F_ID, F_SEL = 0, 1, 2, 3, 20, 37, 46, 174
F_TOT = 190
# bf16 const-group free offsets
B_XS0, B_XS1, B_WT0, B_WT1, B_WB1, B_WB2, B_ONE = 0, 1024, 2048, 2176, 2304, 2321, 2338
B_TOT = 2339

_BUILD_CACHE = {}


def _pin_act_table_set():
    """Restrict walrus's ACT-table choice to natural_log_exp_and_others so
    the kernel's exp/ln mix never ping-pongs table loads."""
    if os.environ.get("BASS_ACT_ROOT_JSON_PATH"):
        return
    try:
        import glob
        import json
        import tempfile

        import neuronxcc

        pwp = os.path.join(os.path.dirname(neuronxcc.__file__), "pwp",
                           "pwp_bin_trainium")
        info = json.load(open(os.path.join(pwp, "act_info.json")))
        keep = [s for s in info["act_func_sets"]
                if s["name"] == "natural_log_exp_and_others"]
        if not keep:
            return
        d = tempfile.mkdtemp(prefix="act_root_")
        for f in glob.glob(os.path.join(pwp, "*")):
            dst = os.path.join(d, os.path.basename(f))
            if not os.path.exists(dst):
                os.symlink(f, dst)
        out = dict(info)
        out["act_func_sets"] = keep
        patched = os.path.join(d, "act_info.json")
        os.unlink(patched)
        with open(patched, "w") as fh:
            json.dump(out, fh)
        import concourse.hw_specs as hw_specs

        tables = {
            keep[0]["name"]: {AF.from_pwp(v) for v in keep[0]["act"].keys()}
        }

        def _tables(arch, _t=tables):
            return _t

        hw_specs.get_activation_tables = _tables
        bacc.get_activation_tables = _tables
        os.environ["BASS_ACT_ROOT_JSON_PATH"] = patched
    except Exception:
        pass


def _heavy_pass(nc, psum, op_sb, rhs_tile, ncols, out_cb, rhs_cols=None):
    """out = Op @ X: Op is a q-chunked lhsT sbuf tile [128, 8q, 8k, 128]
    (bf16); rhs_tile [128, 8, ncols] bf16. Calls out_cb(q, ps[128, ncols])."""
    for q in range(NQ):
        ps = psum.tile([128, ncols], F32, tag="hv")
        for k in range(NQ):
            rhs = rhs_tile[:, k, :ncols] if rhs_cols is None else rhs_cols(k)
            nc.tensor.matmul(
                ps[:],
                lhsT=op_sb[:, q, k, :],
                rhs=rhs,
                start=(k == 0),
                stop=(k == NQ - 1),
            )
        out_cb(q, ps)


def build_bass():
    if "nc" in _BUILD_CACHE:
        return _BUILD_CACHE["nc"]
    _pin_act_table_set()
    nc = bacc.Bacc("TRN2", target_bir_lowering=False, debug=False,
                   num_devices=NCORES)
    dp = nc.declare_dram_parameter

    # --- kernel inputs (per-core host-prepped) ---
    lt = dp("lt", [128, NQ, NQ, 128], BF16, isOutput=False)
    dt = dp("dt", [128, NQ, NQ, 128], BF16, isOutput=False)
    cf32 = dp("cf32", [128, F_TOT], F32, isOutput=False)
    cbf16 = dp("cbf16", [128, B_TOT], BF16, isOutput=False)
    negA1 = dp("negA1", [128, NQ, HS], F32, isOutput=False)
    negA2 = dp("negA2", [128, NQ, HS], F32, isOutput=False)
    mgc = dp("mgc", [NN, 2 * HS], BF16, isOutput=False)
    ids = dp("ids", [128, NQ], I32, isOutput=False)

    c1o = dp("c1o", [128, NQ, HS], F32, isOutput=True)
    c2o = dp("c2o", [128, NQ, HS], F32, isOutput=True)

    # collective bounce buffers (bf16 u2 payload) + warmup dummies
    ag_in = nc.dram_tensor("ag_in", [HS, 1024], BF16)
    ag_out = nc.dram_tensor("ag_out", [128, 1024], BF16, addr_space="Shared")
    agw_in = nc.dram_tensor("agw_in", [1, 32], F32)
    agw_out = nc.dram_tensor("agw_out", [8, 32], F32, addr_space="Shared")

    with tile.TileContext(nc) as tc:
        with tc.tile_pool(name="const", bufs=1) as cst, \
             tc.tile_pool(name="work", bufs=1) as wk, \
             tc.tile_pool(name="psum", bufs=4, space="PSUM") as psum, \
             tc.tile_pool(name="psmall", bufs=2, space="PSUM") as psmall, \
             tc.tile_pool(name="ptrp", bufs=2, space="PSUM") as ptrp:

            # warm the collectives stack: dummy AllGather with no producers,
            # triggers at preamble end and absorbs the first-op barrier cost
            nc.gpsimd.collective_compute(
                "AllGather", OP.bypass,
                replica_groups=[list(range(NCORES))],
                ins=[agw_in[:]], outs=[agw_out[:]],
            )

            # ---------- grouped constant loads ----------
            ids_sb = cst.tile([128, NQ], I32, tag="ids")
            nc.sync.dma_start(out=ids_sb[:], in_=ids[:])
            cf = cst.tile([128, F_TOT], F32, tag="cf")
            nc.sync.dma_start(out=cf[:], in_=cf32[:])
            cb = cst.tile([128, B_TOT], BF16, tag="cb")
            nc.sync.dma_start(out=cb[:], in_=cbf16[:])

            # operator loads: lt on the SP queue, dt (+negA) on the ACT queue
            lt_sb = cst.tile([128, NQ, NQ, 128], BF16, tag="lt")
            dt_sb = cst.tile([128, NQ, NQ, 128], BF16, tag="dt")
            negA_sb = [cst.tile([128, NQ, HS], F32, tag=f"negA{s}",
                                name=f"negA_sb{s}") for s in range(2)]
            nc.scalar.dma_start(out=negA_sb[0][:], in_=negA1[:])
            nc.scalar.dma_start(out=negA_sb[1][:], in_=negA2[:])
            nc.scalar.dma_start(out=dt_sb[:], in_=dt[:])
            nc.sync.dma_start(out=lt_sb[:], in_=lt[:])

            # const views
            btune_v = cf[:, F_BT:F_BT + 1]
            rms_v = [cf[:, F_R1:F_R1 + 1], cf[:, F_R2:F_R2 + 1]]
            bbc_v = [cf[:, F_BB1:F_BB1 + HS + 1], cf[:, F_BB2:F_BB2 + HS + 1]]
            actb_v = cf[:, F_ACT:F_ACT + 9]
            ident_v = cf[:, F_ID:F_ID + 128]
            sel_v = cf[:, F_SEL:F_SEL + HS]
            xs_v = [cb[:, B_XS0:B_XS0 + 1024], cb[:, B_XS1:B_XS1 + 1024]]
            wt_v = [cb[:, B_WT0:B_WT0 + 128], cb[:, B_WT1:B_WT1 + 128]]
            wb_v = [cb[:, B_WB1:B_WB1 + HS + 1], cb[:, B_WB2:B_WB2 + HS + 1]]
            ones_v = cb[:, B_ONE:B_ONE + 1]

            # memory-table gathers (software DGE on gpsimd; combined bf16
            # m1|m2 table: 1024 descriptors x 64B)
            mg = wk.tile([128, NQ, 2 * HS], BF16, tag="mg")
            for q in range(NQ):
                nc.gpsimd.indirect_dma_start(
                    out=mg[:, q, :],
                    out_offset=None,
                    in_=mgc[:],
                    in_offset=bass.IndirectOffsetOnAxis(
                        ap=ids_sb[:, q:q + 1], axis=0),
                )

            # zt^T = W_tune^T @ x_in^T + b_tune   [128 H, 1024 nodes] f32
            ztT = wk.tile([128, 1024], F32, tag="ztT")
            for hhalf in range(2):
                ps = psmall.tile([128, 512], F32, tag="sp")
                cols = slice(hhalf * 512, (hhalf + 1) * 512)
                nc.tensor.matmul(ps[:], lhsT=wt_v[0],
                                 rhs=xs_v[0][:, cols], start=True, stop=False)
                nc.tensor.matmul(ps[:], lhsT=wt_v[1],
                                 rhs=xs_v[1][:, cols], start=False, stop=True)
                nc.vector.tensor_scalar(out=ztT[:, cols], in0=ps[:],
                                        scalar1=btune_v, scalar2=None,
                                        op0=OP.add)

            # zt packed shard [128 node-p, 8q, 16h] via per-core selection
            # transpose: ztp[:, q, :] = (ztT[:, qcols])^T @ I[:, hs]
            ztp = wk.tile([128, NQ, HS], F32, tag="ztp")
            for q in range(NQ):
                pst = ptrp.tile([128, HS], F32, tag="trp")
                nc.tensor.transpose(pst[:], ztT[:, q * 128:(q + 1) * 128],
                                    sel_v)
                nc.vector.tensor_copy(out=ztp[:, q, :], in_=pst[:])

            u2T_full = wk.tile([128, 1024], BF16, tag="u2T_full")
            u2Ts = wk.tile([HS, 1024], BF16, tag="u2Ts")
            c1g = 2.0 * 0.7978845608028654
            c2g = c1g * 0.044715

            couts = (c1o, c2o)

            for s in range(2):  # the two SSM stages
                base = ztT if s == 0 else u2T_full

                # per-q front-end: scaled bf16 lhsT + squares + row-sums
                baseS = wk.tile([128, 1024], BF16, tag=f"baseS{s}")
                sq = wk.tile([128, 1024], BF16, tag=f"sq{s}")
                ssp = wk.tile([128, NQ], F32, tag=f"ssp{s}")
                for q in range(NQ):
                    cols = slice(q * 128, (q + 1) * 128)
                    nc.vector.tensor_scalar(out=baseS[:, cols],
                                            in0=base[:, cols],
                                            scalar1=rms_v[s], scalar2=None,
                                            op0=OP.mult)
                    nc.gpsimd.tensor_tensor(out=sq[:, cols], in0=base[:, cols],
                                            in1=base[:, cols], op=OP.mult)
                    ps = psmall.tile([128, 1], F32, tag="sp")
                    nc.tensor.matmul(ps[:], lhsT=sq[:, cols], rhs=ones_v,
                                     start=True, stop=True)
                    nc.scalar.activation(ssp[:, q:q + 1], ps[:], AF.Copy)
                lnss = wk.tile([128, NQ], F32, tag=f"lnss{s}")
                nc.scalar.activation(lnss[:], ssp[:], AF.Ln)
                rinv = wk.tile([128, NQ], F32, tag=f"rinv{s}")
                nc.scalar.activation(rinv[:], lnss[:], AF.Exp, scale=-0.5,
                                     bias=actb_v[:, 0:1])

                # B/delta matmuls + normalization fold (packed land)
                BD = wk.tile([128, NQ, HS + 1], F32, tag=f"BD{s}")
                for q in range(NQ):
                    ps = psmall.tile([128, HS + 1], F32, tag="sp")
                    nc.tensor.matmul(ps[:], lhsT=baseS[:, q * 128:(q + 1) * 128],
                                     rhs=wb_v[s], start=True, stop=True)
                    nc.vector.scalar_tensor_tensor(
                        out=BD[:, q, :], in0=ps[:], scalar=rinv[:, q:q + 1],
                        in1=bbc_v[s], op0=OP.mult, op1=OP.add)

                # delta = softplus(BD[...,16]) = ln(1+exp(x))
                esp = wk.tile([128, NQ, 1], F32, tag=f"esp{s}")
                nc.scalar.activation(esp[:], BD[:, :, HS:HS + 1], AF.Exp)
                ep1 = wk.tile([128, NQ, 1], F32, tag=f"ep1{s}")
                nc.vector.tensor_scalar(out=ep1[:], in0=esp[:], scalar1=1.0,
                                        scalar2=None, op0=OP.add)
                deltap = wk.tile([128, NQ, 1], F32, tag=f"deltap{s}")
                nc.scalar.activation(deltap[:], ep1[:], AF.Ln)

                # X = B*delta ; dA = delta*negA ; At=exp(dA); M = m_gather*At
                Xf = wk.tile([128, NQ, HS], F32, tag=f"Xf{s}")
                nc.vector.tensor_tensor(
                    out=Xf[:], in0=BD[:, :, 0:HS],
                    in1=deltap[:].to_broadcast([128, NQ, HS]), op=OP.mult)
                dA = wk.tile([128, NQ, HS], F32, tag=f"dA{s}")
                nc.vector.tensor_tensor(
                    out=dA[:], in0=deltap[:].to_broadcast([128, NQ, HS]),
                    in1=negA_sb[s][:], op=OP.mult)
                At = wk.tile([128, NQ, HS], F32, tag=f"At{s}")
                nc.scalar.activation(At[:], dA[:], AF.Exp)
                Mf = wk.tile([128, NQ, HS], F32, tag=f"Mf{s}")
                nc.gpsimd.tensor_tensor(out=Mf[:],
                                        in0=mg[:, :, s * HS:(s + 1) * HS],
                                        in1=At[:], op=OP.mult)

                # moments S0,S1,S2 (need dA only; overlap heavy passes)
                S0 = wk.tile([128, NQ, HS], F32, tag=f"S0{s}")
                S1 = wk.tile([128, NQ, HS], F32, tag=f"S1{s}")
                S2 = wk.tile([128, NQ, HS], F32, tag=f"S2{s}")
                for k in range(8):
                    wE = wk.tile([128, NQ, HS], F32, tag=f"wE{s}_{k % 2}",
                                 name=f"wE{s}_{k}")
                    nc.scalar.activation(wE[:], dA[:], AF.Exp,
                                         scale=float(T_NODES[k]),
                                         bias=actb_v[:, k + 1:k + 2])
                    tk = float(T_NODES[k])
                    if k == 0:
                        nc.vector.tensor_copy(out=S0[:], in_=wE[:])
                        nc.vector.tensor_scalar(out=S1[:], in0=wE[:], scalar1=tk,
                                                scalar2=None, op0=OP.mult)
                        nc.vector.tensor_scalar(out=S2[:], in0=wE[:],
                                                scalar1=tk * tk, scalar2=None,
                                                op0=OP.mult)
                    else:
                        nc.vector.tensor_tensor(out=S0[:], in0=S0[:], in1=wE[:],
                                                op=OP.add)
                        nc.vector.scalar_tensor_tensor(
                            out=S1[:], in0=wE[:], scalar=tk, in1=S1[:],
                            op0=OP.mult, op1=OP.add)
                        nc.vector.scalar_tensor_tensor(
                            out=S2[:], in0=wE[:], scalar=tk * tk, in1=S2[:],
                            op0=OP.mult, op1=OP.add)

                # bf16 rhs group for pass L1: R0 = [X | M]
                R0 = wk.tile([128, NQ, 2 * HS], BF16, tag=f"R0{s}")
                nc.scalar.activation(R0[:, :, 0:HS], Xf[:], AF.Copy)
                nc.vector.tensor_copy(out=R0[:, :, HS:2 * HS], in_=Mf[:])

                # ---- heavy pass L1: L @ [X | M] -> LX, Y1 ----
                R1 = wk.tile([128, NQ, 3 * HS], BF16, tag=f"R1{s}")  # [V|M|Y1]
                nc.vector.tensor_copy(out=R1[:, :, HS:2 * HS],
                                      in_=R0[:, :, HS:2 * HS])

                def l1_cb(q, ps, R1=R1, Xf=Xf):
                    # V = X - REG*LX  (bf16 into R1) ; Y1 = psum[:,16:32]
                    nc.vector.scalar_tensor_tensor(
                        out=R1[:, q, 0:HS], in0=ps[:, 0:HS], scalar=-REG,
                        in1=Xf[:, q, :], op0=OP.mult, op1=OP.add)
                    nc.scalar.activation(R1[:, q, 2 * HS:3 * HS],
                                         ps[:, HS:2 * HS], AF.Copy)

                _heavy_pass(nc, psum, lt_sb, R0, 2 * HS, l1_cb)

                # ---- heavy pass D1: D @ [V | M | Y1] -> U, UM, T1 ----
                # acc_q = M - REG*UM + REG^2*T1 computed here from psum
                R2 = wk.tile([128, NQ, 3 * HS], BF16, tag=f"R2{s}")  # [W1|U|UM]
                acc = wk.tile([128, NQ, HS], F32, tag=f"acc{s}")

                def d1_cb(q, ps, R2=R2, acc=acc, Mf=Mf):
                    nc.scalar.activation(R2[:, q, HS:3 * HS], ps[:, 0:2 * HS],
                                         AF.Copy)
                    nc.vector.scalar_tensor_tensor(
                        out=acc[:, q, :], in0=ps[:, HS:2 * HS], scalar=-REG,
                        in1=Mf[:, q, :], op0=OP.mult, op1=OP.add)
                    nc.vector.scalar_tensor_tensor(
                        out=acc[:, q, :], in0=ps[:, 2 * HS:3 * HS], scalar=REG2,
                        in1=acc[:, q, :], op0=OP.mult, op1=OP.add)

                _heavy_pass(nc, psum, dt_sb, R1, 3 * HS, d1_cb)

                # ---- heavy pass L2: L @ V -> W1 ----
                def l2_cb(q, ps, R2=R2):
                    nc.scalar.activation(R2[:, q, 0:HS], ps[:, 0:HS], AF.Copy)

                _heavy_pass(nc, psum, lt_sb, R1, HS, l2_cb)

                # ---- heavy pass D2: D @ [W1 | U | UM] -> P, Q, T2 ----
                # per-q tail: finish combine; stage 0 also gelu+u2+transpose
                tV = wk.tile([128, NQ, HS], F32, tag=f"tV{s}")
                tU = wk.tile([128, NQ, HS], F32, tag=f"tU{s}")
                tP = wk.tile([128, NQ, HS], F32, tag=f"tP{s}")
                tQ = wk.tile([128, NQ, HS], F32, tag=f"tQ{s}")
                if s == 0:
                    g1 = wk.tile([128, NQ, HS], F32, tag="g1")
                    g2 = wk.tile([128, NQ, HS], F32, tag="g2")
                    u2p = wk.tile([128, NQ, HS], F32, tag="u2p")

                def d2_cb(q, ps, s=s, acc=acc, R1=R1, R2=R2,
                          S0=S0, S1=S1, S2=S2):
                    aq = acc[:, q, :]
                    # ps = [P | Q | T2]
                    nc.vector.scalar_tensor_tensor(
                        out=aq, in0=ps[:, 2 * HS:3 * HS], scalar=REG2 / 2,
                        in1=aq, op0=OP.mult, op1=OP.add)
                    nc.vector.tensor_tensor(out=tP[:, q, :], in0=ps[:, 0:HS],
                                            in1=S1[:, q, :], op=OP.mult)
                    nc.vector.tensor_tensor(out=tQ[:, q, :],
                                            in0=ps[:, HS:2 * HS],
                                            in1=S2[:, q, :], op=OP.mult)
                    nc.gpsimd.tensor_tensor(out=tV[:, q, :],
                                            in0=R1[:, q, 0:HS],
                                            in1=S0[:, q, :], op=OP.mult)
                    nc.gpsimd.tensor_tensor(out=tU[:, q, :],
                                            in0=R2[:, q, HS:2 * HS],
                                            in1=S1[:, q, :], op=OP.mult)
                    nc.vector.tensor_tensor(out=aq, in0=aq, in1=tV[:, q, :],
                                            op=OP.add)
                    nc.vector.scalar_tensor_tensor(
                        out=aq, in0=tU[:, q, :], scalar=-REG, in1=aq,
                        op0=OP.mult, op1=OP.add)
                    nc.vector.scalar_tensor_tensor(
                        out=aq, in0=tP[:, q, :], scalar=REG2, in1=aq,
                        op0=OP.mult, op1=OP.add)
                    nc.vector.scalar_tensor_tensor(
                        out=aq, in0=tQ[:, q, :], scalar=REG2 / 2, in1=aq,
                        op0=OP.mult, op1=OP.add)
                    if s == 0:
                        # u2 = zt + gelu(c1) per q, then transpose to u2Ts
                        g1q, g2q, u2q = g1[:, q, :], g2[:, q, :], u2p[:, q, :]
                        nc.gpsimd.tensor_tensor(out=g1q, in0=aq, in1=aq,
                                                op=OP.mult)
                        nc.vector.tensor_scalar(out=g1q, in0=g1q,
                                                scalar1=-c2g, scalar2=-c1g,
                                                op0=OP.mult, op1=OP.add)
                        nc.gpsimd.tensor_tensor(out=g2q, in0=aq, in1=g1q,
                                                op=OP.mult)
                        nc.scalar.activation(g2q, g2q, AF.Exp)
                        nc.vector.tensor_scalar(out=g2q, in0=g2q, scalar1=1.0,
                                                scalar2=None, op0=OP.add)
                        nc.scalar.activation(g2q, g2q, AF.Ln)
                        nc.scalar.activation(g2q, g2q, AF.Exp, scale=-1.0)
                        nc.gpsimd.tensor_tensor(out=u2q, in0=aq, in1=g2q,
                                                op=OP.mult)
                        nc.vector.tensor_tensor(out=u2q, in0=ztp[:, q, :],
                                                in1=u2q, op=OP.add)
                        pst = ptrp.tile([HS, 128], F32, tag="trp")
                        nc.tensor.transpose(pst[:], u2q, ident_v)
                        nc.vector.tensor_copy(
                            out=u2Ts[:, q * 128:(q + 1) * 128], in_=pst[:])

                _heavy_pass(nc, psum, dt_sb, R2, 3 * HS, d2_cb)

                if s == 0:
                    nc.sync.dma_start(out=ag_in[:], in_=u2Ts[:])
                    nc.gpsimd.collective_compute(
                        "AllGather", OP.bypass,
                        replica_groups=[list(range(NCORES))],
                        ins=[ag_in[:]], outs=[ag_out[:]],
                    )
                    nc.sync.dma_start(out=u2T_full[:], in_=ag_out[:])
                # write output shard (after ag_in on the SP queue for s=0)
                nc.sync.dma_start(out=couts[s][:], in_=acc[:])

    nc.compile()
    _BUILD_CACHE["nc"] = nc
    return nc


def _pack_q(a_T):
    """[1024, 1024] transposed operator -> [128, 8q, 8k, 128] bf16,
    element [p, q, k, c] = a_T[k*128+p, q*128+c]."""
    r = a_T.reshape(NQ, 128, NQ, 128).transpose(1, 2, 0, 3)
    return np.ascontiguousarray(r).astype(BF)


def kernel(**inputs):
    out, _ = _run(inputs, trace=False)
    return out


def _run(inputs, trace=False, trace_kwargs=None):
    inp = {k: np.asarray(v) for k, v in inputs.items()}
    L = inp["L_agg"].astype(np.float32)
    D = inp["delta_L_agg"].astype(np.float32)
    x_sub = inp["x_sub"].astype(np.float32)
    m1 = inp["m1_vec"].astype(np.float32)
    m2 = inp["m2_vec"].astype(np.float32)
    names = inp["names_table"].astype(np.float32)
    rms1 = inp["rms1_scale"].astype(np.float32)
    rms2 = inp["rms2_scale"].astype(np.float32)
    W_tune = inp["W_tune"].astype(np.float32)
    b_tune = inp["b_tune"].astype(np.float32)
    W_B1 = inp["W_B1"].astype(np.float32)
    b_B1 = inp["b_B1"].astype(np.float32)
    W_B2 = inp["W_B2"].astype(np.float32)
    b_B2 = inp["b_B2"].astype(np.float32)
    W_dt = inp["W_dt"].astype(np.float32)
    b_dt = inp["b_dt"].astype(np.float32)
    A1 = inp["A_log_1"].astype(np.float32)
    A2 = inp["A_log_2"].astype(np.float32)
    tsrc = np.asarray(inp["target_src"]).astype(np.int64)
    tdst = np.asarray(inp["target_dst"]).astype(np.int64)
    aids = np.asarray(inp["active_input_ids"]).astype(np.int64)

    # x_in = [x_sub | neigh]; the names_table neighbor embedding (ED=1)
    neigh = np.zeros((NA, 2 * ED), np.float32)
    neigh[:E, :ED] = names[tsrc]
    neigh[:E, ED:] = names[tdst]
    neigh[E:2 * E, :ED] = names[tdst]
    neigh[E:2 * E, ED:] = names[tsrc]
    x_in = np.concatenate([x_sub, neigh], axis=1)  # [1024, 174]
    xsT = np.zeros((KDP, NA), np.float32)
    xsT[:KD] = x_in.T
    wtune_p = np.zeros((KDP, H), np.float32)
    wtune_p[:KD] = W_tune

    lt_p = _pack_q(np.ascontiguousarray(L.T))
    dt_p = _pack_q(np.ascontiguousarray(D.T))

    ids_p = np.ascontiguousarray(
        aids.astype(np.int32).reshape(NQ, 128).T)  # [128p, 8q]

    negA1_full = -np.exp(A1)  # [128]
    negA2_full = -np.exp(A2)

    eye = np.eye(128, dtype=np.float32)
    actb = np.array([0.5 * np.log(H)] + [np.log(w) for w in T_W], np.float32)

    cbf = np.zeros((128, B_TOT), BF)
    cbf[:, B_XS0:B_XS0 + 1024] = xsT[:128].astype(BF)
    cbf[:, B_XS1:B_XS1 + 1024] = xsT[128:].astype(BF)
    cbf[:, B_WT0:B_WT0 + 128] = wtune_p[:128].astype(BF)
    cbf[:, B_WT1:B_WT1 + 128] = wtune_p[128:].astype(BF)
    cbf[:, B_ONE] = np.ones(128, BF)

    common = {
        "lt": lt_p, "dt": dt_p, "ids": ids_p,
    }

    in_maps = []
    for c in range(NCORES):
        hs = slice(c * HS, (c + 1) * HS)
        cfc = np.zeros((128, F_TOT), np.float32)
        cfc[:, F_BT] = b_tune
        cfc[:, F_R1] = rms1
        cfc[:, F_R2] = rms2
        cfc[:, F_BB1:F_BB1 + HS + 1] = np.concatenate([b_B1[hs], b_dt])
        cfc[:, F_BB2:F_BB2 + HS + 1] = np.concatenate([b_B2[hs], b_dt])
        cfc[:, F_ACT:F_ACT + 9] = actb
        cfc[:, F_ID:F_ID + 128] = eye
        cfc[:, F_SEL:F_SEL + HS] = eye[:, hs]
        cbc = cbf.copy()
        cbc[:, B_WB1:B_WB1 + HS + 1] = np.concatenate(
            [W_B1[:, hs], W_dt], axis=1).astype(BF)
        cbc[:, B_WB2:B_WB2 + HS + 1] = np.concatenate(
            [W_B2[:, hs], W_dt], axis=1).astype(BF)
        nA1 = np.tile(negA1_full[hs], (128, NQ, 1)).astype(np.float32)
        nA2 = np.tile(negA2_full[hs], (128, NQ, 1)).astype(np.float32)
        in_maps.append({
            **common,
            "cf32": cfc, "cbf16": cbc,
            "negA1": nA1, "negA2": nA2,
            "mgc": np.ascontiguousarray(
                np.concatenate([m1[:, hs], m2[:, hs]], axis=1)).astype(BF),
        })

    nc = build_bass()
    res = run_bass_kernel_spmd(nc, in_maps, core_ids=list(range(NCORES)),
                               trace=trace, **(trace_kwargs or {}))

    out = np.zeros((2, NA, H), np.float32)
    for c in range(NCORES):
        hs = slice(c * HS, (c + 1) * HS)
        # packed [128p, 8q, 16h] -> [1024, 16]
        out[0][:, hs] = res.results[c]["c1o"].transpose(1, 0, 2).reshape(NA, HS)
        out[1][:, hs] = res.results[c]["c2o"].transpose(1, 0, 2).reshape(NA, HS)
    return out, res
